# revision 1
# baseline (speedup 1.0000x reference)
"""MultiHeadEMA Trainium2 Bass kernel.

Reference computation (B=4, S=8192, D=1024, N=2):
    out = silu(conv_causal(x, k) + x * omega)
    k[d, l] = sum_n c[d, n] * q[d, n]^l
    q = 1 - sigmoid(delta) * sigmoid(alpha)
    c = sigmoid(delta) * beta * gamma * sqrt(1/N)

The length-S causal conv with a sum-of-2-exponentials kernel is a pair of
first-order linear recurrences (EMA scans):
    h_n[t] = q_n * h_n[t-1] + x[t]
    y[t]   = c_1 h_1[t] + c_2 h_2[t]
    out[t] = silu(y[t] + omega * x[t])

Sharding: D=1024 split across 8 cores (128 channels each).  Each core works
in [channel-partition, time-free] layout; the scans run on the Vector engine
via TensorTensorScanArith, one recurrence per partition.  The host transposes
x to [B, D, S] while slicing the per-core shards and transposes the per-core
results back while gathering (part of the shard/unshard contract).
"""

import math

import numpy as np

import concourse.bass as bass
import concourse.mybir as mybir
import concourse.tile as tile
from concourse import bacc
from concourse.bass_utils import run_bass_kernel_spmd

B = 4
S = 8192
D = 1024
N_CORES = 8
D_LOC = D // N_CORES  # 128 channels per core
SCALE = math.sqrt(1.0 / 2.0)

F32 = mybir.dt.float32


def build_nc(b=B, d_loc=D_LOC, s=S, t_chunk=2048, act="Silu",
             x_bufs=3, h_bufs=3, tmp_bufs=3, acc_bufs=2):
    """Build the per-core Bass module (SPMD: same NEFF on all cores).

    Inputs (per core):
      x  [b, d_loc, s] f32 — time-major-last shard of the input
      pp [d_loc, 8]    f32 — packed params: q1 q2 c1 c2 w (cols 0-4)
    Output:
      o  [b, d_loc, s] f32
    """
    assert s % t_chunk == 0
    n_chunks = s // t_chunk
    # Non-uniform chunk schedule: small chunks at the very start (fill the
    # pipeline quickly) and at the very end (short drain tail).  Middle runs
    # at full t_chunk.  Only the first/last batch get the ramps.
    def chunk_schedule(bi):
        full = [t_chunk] * n_chunks
        ramp = [t_chunk // 8, t_chunk // 8, t_chunk // 4, t_chunk // 2]
        if bi == 0 and n_chunks >= 2:
            return ramp + [t_chunk] * (n_chunks - 1)
        if bi == b - 1 and n_chunks >= 2:
            return [t_chunk] * (n_chunks - 1) + ramp[::-1]
        return full

    nc = bacc.Bacc(
        "TRN2",
        target_bir_lowering=False,
        debug=False,
        enable_asserts=False,
        num_devices=N_CORES,
    )

    x_d = nc.dram_tensor("x", [b, d_loc, s], F32, kind="ExternalInput").ap()
    pp_d = nc.dram_tensor("pp", [d_loc, 12], F32, kind="ExternalInput").ap()
    o_d = nc.dram_tensor("o", [b, d_loc, s], F32, kind="ExternalOutput").ap()

    with tile.TileContext(nc) as tc:
        with (
            tc.tile_pool(name="pp", bufs=1) as pp_pool,
            tc.tile_pool(name="x", bufs=x_bufs) as x_pool,
            tc.tile_pool(name="h", bufs=h_bufs) as h_pool,
            tc.tile_pool(name="tmp", bufs=tmp_bufs) as tmp_pool,
            tc.tile_pool(name="acc", bufs=acc_bufs) as acc_pool,
        ):
            # pp rides the GpSimd SWDGE path so the HWDGE queue's first
            # (cold, ~3us setup) transfer is the first x chunk itself —
            # overlapping the two queue spin-ups at kernel start.
            pp = pp_pool.tile([d_loc, 12], F32, tag="pp")
            nc.gpsimd.dma_start(out=pp[:], in_=pp_d[:])
            q1 = pp[:, 0:1]
            q2 = pp[:, 1:2]
            c1 = pp[:, 2:3]
            c2 = pp[:, 3:4]
            w = pp[:, 4:5]
            q1sq_b = pp[:, 5:6].broadcast_to([d_loc, t_chunk // 2])
            q2sq_b = pp[:, 6:7].broadcast_to([d_loc, t_chunk // 2])
            c1q1 = pp[:, 7:8]
            c2q2 = pp[:, 8:9]
            ccw = pp[:, 9:10]

            mult = mybir.AluOpType.mult
            add = mybir.AluOpType.add
            COPY = mybir.ActivationFunctionType.Copy
            ACT = getattr(mybir.ActivationFunctionType, act)

            # Radix-2 polyphase: the time-major scan halves its length by
            # scanning only even positions (h_e[m] = q^2 h_e[m-1] + u[m],
            # u[m] = q*x[2m-1] + x[2m]); odd positions never materialize —
            # they fold into the combine as r_odd = c1q1*h1e + c2q2*h2e +
            # (c1+c2+w)*x_odd.  Strided SBUF access is full-rate on both
            # DVE and ACT (measured), so only the scan shrinks.
            h1_prev = None
            h2_prev = None
            for bi in range(b):
                t0 = 0
                for j, tc_len in enumerate(chunk_schedule(bi)):
                    m = tc_len // 2
                    # x tile with 1-element halo in column 0 (= x[t0-1])
                    xt = x_pool.tile([d_loc, t_chunk + 2], F32, tag="x")
                    if j == 0:
                        nc.vector.memset(xt[:, 0:1], 0.0)
                        nc.sync.dma_start(
                            out=xt[:, 1 : tc_len + 1],
                            in_=x_d[bi, :, t0 : t0 + tc_len],
                        )
                    else:
                        nc.sync.dma_start(
                            out=xt[:, 0 : tc_len + 1],
                            in_=x_d[bi, :, t0 - 1 : t0 + tc_len],
                        )
                    # phase views (columns: [halo, x0, x1, ..., x_{T-1}, pad])
                    xop = xt[:, 0 : 2 * m].rearrange("p (m two) -> p m two", two=2)[:, :, 0]   # x[2m-1]
                    xe = xt[:, 1 : 2 * m + 1].rearrange("p (m two) -> p m two", two=2)[:, :, 0]  # x[2m]
                    xo = xt[:, 2 : 2 * m + 2].rearrange("p (m two) -> p m two", two=2)[:, :, 0]  # x[2m+1]

                    # u_n = q_n * x[2m-1] + x[2m]    (Vector)
                    u1 = acc_pool.tile([d_loc, t_chunk // 2], F32, tag="u1")
                    u2 = acc_pool.tile([d_loc, t_chunk // 2], F32, tag="u2")
                    nc.vector.scalar_tensor_tensor(u1[:, :m], xop, q1, xe, mult, add)
                    nc.vector.scalar_tensor_tensor(u2[:, :m], xop, q2, xe, mult, add)

                    # residual pre-scales on the Scalar engine
                    t1e = tmp_pool.tile([d_loc, t_chunk // 2], F32, tag="t1e")
                    t1o = tmp_pool.tile([d_loc, t_chunk // 2], F32, tag="t1o")
                    nc.scalar.activation(t1e[:, :m], xe, COPY, scale=w)
                    nc.scalar.activation(t1o[:, :m], xo, COPY, scale=ccw)

                    # half-length scans over even positions (chained)
                    i1 = 0.0 if j == 0 else h1_prev
                    i2 = 0.0 if j == 0 else h2_prev
                    h1 = h_pool.tile([d_loc, t_chunk // 2], F32, tag="h1")
                    h2 = h_pool.tile([d_loc, t_chunk // 2], F32, tag="h2")
                    nc.vector.tensor_tensor_scan(
                        h1[:, :m], q1sq_b[:, :m], u1[:, :m], i1, mult, add
                    )
                    nc.vector.tensor_tensor_scan(
                        h2[:, :m], q2sq_b[:, :m], u2[:, :m], i2, mult, add
                    )
                    h1_prev = h1[:, m - 1 : m]
                    h2_prev = h2[:, m - 1 : m]

                    # combines (Vector, fused muladds)
                    ue = acc_pool.tile([d_loc, t_chunk // 2], F32, tag="ue")
                    re = acc_pool.tile([d_loc, t_chunk // 2], F32, tag="re")
                    nc.vector.scalar_tensor_tensor(
                        ue[:, :m], h1[:, :m], c1, t1e[:, :m], mult, add
                    )
                    nc.vector.scalar_tensor_tensor(
                        re[:, :m], h2[:, :m], c2, ue[:, :m], mult, add
                    )
                    uo = acc_pool.tile([d_loc, t_chunk // 2], F32, tag="uo")
                    ro = acc_pool.tile([d_loc, t_chunk // 2], F32, tag="ro")
                    nc.vector.scalar_tensor_tensor(
                        uo[:, :m], h1[:, :m], c1q1, t1o[:, :m], mult, add
                    )
                    nc.vector.scalar_tensor_tensor(
                        ro[:, :m], h2[:, :m], c2q2, uo[:, :m], mult, add
                    )

                    # silu with interleaving strided writes (Scalar)
                    ot = tmp_pool.tile([d_loc, t_chunk], F32, tag="ot")
                    ot2 = ot[:, : 2 * m].rearrange("p (m two) -> p m two", two=2)
                    nc.scalar.activation(ot2[:, :, 0], re[:, :m], ACT)
                    nc.scalar.activation(ot2[:, :, 1], ro[:, :m], ACT)
                    nc.sync.dma_start(
                        out=o_d[bi, :, t0 : t0 + tc_len], in_=ot[:, :tc_len]
                    )
                    t0 += tc_len
                assert t0 == s

    nc.compile()
    return nc


def _host_params(delta, alpha, beta, gamma, omega):
    """Compute per-channel scan params on the host (O(D*N) work)."""
    p = 1.0 / (1.0 + np.exp(-delta[:, :, 0].astype(np.float64)))  # [D, N]
    a = 1.0 / (1.0 + np.exp(-alpha[:, :, 0].astype(np.float64)))
    q = 1.0 - p * a                                               # [D, N]
    c = p * beta[:, :, 0].astype(np.float64) * gamma.astype(np.float64) * SCALE
    pp = np.zeros((D, 12), dtype=np.float32)
    pp[:, 0] = q[:, 0]
    pp[:, 1] = q[:, 1]
    pp[:, 2] = c[:, 0]
    pp[:, 3] = c[:, 1]
    pp[:, 4] = omega
    pp[:, 5] = q[:, 0] ** 2
    pp[:, 6] = q[:, 1] ** 2
    pp[:, 7] = c[:, 0] * q[:, 0]
    pp[:, 8] = c[:, 1] * q[:, 1]
    pp[:, 9] = c[:, 0] + c[:, 1] + omega
    return pp


_NC_CACHE = {}


def kernel(x, delta, alpha, beta, gamma, omega):
    x = np.asarray(x, dtype=np.float32)
    delta = np.asarray(delta, dtype=np.float32)
    alpha = np.asarray(alpha, dtype=np.float32)
    beta = np.asarray(beta, dtype=np.float32)
    gamma = np.asarray(gamma, dtype=np.float32)
    omega = np.asarray(omega, dtype=np.float32)
    assert x.shape == (B, S, D)

    if "nc" not in _NC_CACHE:
        _NC_CACHE["nc"] = build_nc(
            t_chunk=4096, x_bufs=2, h_bufs=2, tmp_bufs=2, acc_bufs=1
        )
    nc = _NC_CACHE["nc"]

    pp = _host_params(delta, alpha, beta, gamma, omega)
    xt = np.ascontiguousarray(x.transpose(0, 2, 1))  # [B, D, S]

    in_maps = []
    for i in range(N_CORES):
        sl = slice(i * D_LOC, (i + 1) * D_LOC)
        in_maps.append(
            {
                "x": np.ascontiguousarray(xt[:, sl, :]),
                "pp": np.ascontiguousarray(pp[sl]),
            }
        )

    res = run_bass_kernel_spmd(nc, in_maps, core_ids=list(range(N_CORES)))

    out = np.empty((B, S, D), dtype=np.float32)
    for i in range(N_CORES):
        sl = slice(i * D_LOC, (i + 1) * D_LOC)
        out[:, :, sl] = res.results[i]["o"].transpose(0, 2, 1)
    return out



# revision 5
# speedup vs baseline: 1.7755x; 1.7755x over previous
"""MultiHeadEMA Trainium2 Bass kernel (radix-8 blocked scan, matmul-offloaded).

Reference computation (B=4, S=8192, D=1024, N=2):
    out = silu(conv_causal(x, k) + x * omega)
    k[d, l] = sum_n c[d, n] * q[d, n]^l
    q = 1 - sigmoid(delta) * sigmoid(alpha)
    c = sigmoid(delta) * beta * gamma * sqrt(1/N)

The causal conv is a pair of first-order recurrences per channel.  On TRN2
the per-partition-scalar DVE ops (scalar_tensor_tensor / tensor_tensor_scan)
run at 1 elem/cycle with no fast modes, so the baseline that ran everything
on the Vector engine was Vector-bound at ~94% busy.  This version blocks the
recurrence by J=8 timesteps and restructures all the muladd work as
cross-partition matmuls on the otherwise-idle Tensor engine:

  - layout: partition p = (phase j in [0,8), channel c in [0,16)) per group
    of 16 channels; 8 groups cover the core's 128 channels; free dim is the
    block index m in [0, S/8).
  - u[m]   = sum_j q^{7-j} x[8m+j]          -> one matmul pass (weights wu)
  - h[m]   = q^8 h[m-1] + u[m]              -> DVE scan, 8x shorter, both
             recurrences x 4 groups stacked in one 128-partition scan
  - y[8m+j] = sum_n c_n q_n^{j+1} h_n[m-1]  -> matmul pass (weights wv)
            + sum_{i<=j} g_{j-i} x[8m+i] + w x[8m+j]   -> matmul pass (wx)
  - out = silu(y) fused on the Scalar engine, PSUM -> fp16 SBUF.

fp16 end-to-end I/O halves DMA traffic (8 MiB in + 8 MiB out per core);
weights are host-computed in fp64 and shipped as fp16 (the scan multiplier
q^8 stays fp32; the scan state is fp32 internally).  Numpy sim of this exact
quantization measures rel err 6.7e-4 vs the fp32 reference.

Sharding: D=1024 split across 8 cores (128 channels each); host packs the
phase-major fp16 layout and unpacks the result (part of shard/unshard).
"""

import math

import numpy as np

import concourse.bass as bass
import concourse.mybir as mybir
import concourse.tile as tile
from concourse import bacc
from concourse.bass_utils import run_bass_kernel_spmd

B = 4
S = 8192
D = 1024
N = 2
N_CORES = 8
D_LOC = D // N_CORES      # 128 channels per core
J = 8                     # timesteps per block (radix)
C = 16                    # channels per group
G = D_LOC // C            # 8 groups
M = S // J                # 1024 blocks per batch
SCALE = math.sqrt(1.0 / N)

F32 = mybir.dt.float32
F16 = mybir.dt.float16


def build_nc(x_bufs=2, o_bufs=2, h_bufs=2, y_bufs=4, act="Silu"):
    """Per-core Bass module (SPMD: same NEFF on all cores).

    Inputs (per core):
      x  [B, 128, G*M] f16 — phase-major shard: x[b, j*16+c, g*M+m]
                              = x_orig[b, t=8m+j, ch=16g+c]
      wu [128, G*32]   f16 — u-prep weights, lhsT per group
      wv [128, G*128]  f16 — h-combine weights, lhsT per group
      wx [128, G*128]  f16 — x-combine weights, lhsT per group
      a  [128, 2]      f32 — scan multipliers q^8 for stacked tiles A, B
    Output:
      o  [B, 128, G*M] f16 — same layout as x
    """
    nc = bacc.Bacc(
        "TRN2",
        target_bir_lowering=False,
        debug=False,
        enable_asserts=False,
        num_devices=N_CORES,
    )

    x_d = nc.dram_tensor("x", [B, 128, G * M], F16, kind="ExternalInput").ap()
    wu_d = nc.dram_tensor("wu", [128, G * 32], F16, kind="ExternalInput").ap()
    wv_d = nc.dram_tensor("wv", [128, G * 128], F16, kind="ExternalInput").ap()
    wx_d = nc.dram_tensor("wx", [128, G * 128], F16, kind="ExternalInput").ap()
    a_d = nc.dram_tensor("a", [128, 2], F32, kind="ExternalInput").ap()
    o_d = nc.dram_tensor("o", [B, 128, G * M], F16, kind="ExternalOutput").ap()

    mult = mybir.AluOpType.mult
    add = mybir.AluOpType.add
    ACT = getattr(mybir.ActivationFunctionType, act)

    with tile.TileContext(nc) as tc:
        with (
            tc.tile_pool(name="w", bufs=1) as w_pool,
            tc.tile_pool(name="x", bufs=x_bufs) as x_pool,
            tc.tile_pool(name="o", bufs=o_bufs) as o_pool,
            tc.tile_pool(name="h", bufs=h_bufs) as h_pool,
            tc.tile_pool(name="u", bufs=1, space="PSUM") as u_pool,
            tc.tile_pool(name="y", bufs=y_bufs, space="PSUM") as y_pool,
        ):
            # params ride the GpSimd SWDGE path so the HWDGE queue's first
            # (cold) transfer is the first x batch itself.
            wu_t = w_pool.tile([128, G * 32], F16, tag="wu")
            wv_t = w_pool.tile([128, G * 128], F16, tag="wv")
            wx_t = w_pool.tile([128, G * 128], F16, tag="wx")
            a_t = w_pool.tile([128, 2], F32, tag="a")
            nc.gpsimd.dma_start(out=wu_t[:], in_=wu_d[:])
            nc.gpsimd.dma_start(out=wv_t[:], in_=wv_d[:])
            nc.gpsimd.dma_start(out=wx_t[:], in_=wx_d[:])
            nc.gpsimd.dma_start(out=a_t[:], in_=a_d[:])

            for b in range(B):
                xb = x_pool.tile([128, G * M], F16, tag="x")
                nc.sync.dma_start(out=xb[:], in_=x_d[b])

                # u-prep: one matmul per (group, 512-col chunk) into the
                # group's 32-partition slice of the stacked PSUM tile.
                u_t = [
                    u_pool.tile([128, M], F32, tag="uA", name="uA"),
                    u_pool.tile([128, M], F32, tag="uB", name="uB"),
                ]
                for g in range(G):
                    tidx, sub = divmod(g, 4)
                    for ck in range(M // 512):
                        nc.tensor.matmul(
                            u_t[tidx][sub * 32 : sub * 32 + 32,
                                      ck * 512 : (ck + 1) * 512],
                            lhsT=wu_t[:, g * 32 : (g + 1) * 32],
                            rhs=xb[:, g * M + ck * 512 : g * M + (ck + 1) * 512],
                            start=True,
                            stop=True,
                            tile_position=(0, sub * 32),
                        )

                # stacked scans: h[m] = q^8 h[m-1] + u[m], fp32 state,
                # fp16 stored h; col 0 holds h[-1] = 0 (batches are
                # independent sequences).
                h_t = [
                    h_pool.tile([128, M + 1], F16, tag="hA", name="hA"),
                    h_pool.tile([128, M + 1], F16, tag="hB", name="hB"),
                ]
                for t in range(2):
                    nc.vector.memset(h_t[t][:, 0:1], 0.0)
                    nc.vector.tensor_tensor_scan(
                        h_t[t][:, 1 : M + 1],
                        a_t[:, t : t + 1].broadcast_to([128, M]),
                        u_t[t][:, :],
                        0.0,
                        mult,
                        add,
                    )

                # combines + fused silu
                ob = o_pool.tile([128, G * M], F16, tag="o")
                for g in range(G):
                    hv = h_t[g // 4]
                    for ck in range(M // 512):
                        y = y_pool.tile([128, 512], F32, tag="y")
                        nc.tensor.matmul(
                            y[:],
                            lhsT=wv_t[:, g * 128 : (g + 1) * 128],
                            rhs=hv[:, ck * 512 : (ck + 1) * 512],
                            start=True,
                            stop=False,
                        )
                        nc.tensor.matmul(
                            y[:],
                            lhsT=wx_t[:, g * 128 : (g + 1) * 128],
                            rhs=xb[:, g * M + ck * 512 : g * M + (ck + 1) * 512],
                            start=False,
                            stop=True,
                        )
                        nc.scalar.activation(
                            ob[:, g * M + ck * 512 : g * M + (ck + 1) * 512],
                            y[:],
                            ACT,
                        )
                nc.scalar.dma_start(out=o_d[b], in_=ob[:])

    nc.compile()
    return nc


def _host_params(delta, alpha, beta, gamma, omega, sl):
    """Per-core weight construction (channel slice sl; fp64 math)."""
    d = delta[sl, :, 0].astype(np.float64)
    al = alpha[sl, :, 0].astype(np.float64)
    p = 1.0 / (1.0 + np.exp(-d))
    aa = 1.0 / (1.0 + np.exp(-al))
    q = 1.0 - p * aa                                     # [128, N]
    c = p * beta[sl, :, 0].astype(np.float64) * gamma[sl].astype(np.float64) * SCALE
    w = omega[sl].astype(np.float64)                     # [128]
    ch = np.arange(D_LOC).reshape(G, C)                  # ch[g, cc] = 16g+cc

    qp = q[:, :, None] ** np.arange(J + 2)[None, None, :]   # [128, N, J+2]

    # wu[g][(j,cc'), n*16+cc] = q_n(ch)^{7-j} delta_{cc,cc'}
    wu = np.zeros((G, 128, 2 * C))
    # wv[g][(sub,n,cc'), (j,cc)] = delta_{sub,g%4} c_n q_n^{j+1}
    wv = np.zeros((G, 128, 128))
    # wx[g][(i,cc'), (j,cc)] = i<j: g_{j-i};  i==j: g_0 + w
    g_r = np.einsum("dn,dnr->dr", c, qp)                 # [128, J+2]
    wx = np.zeros((G, 128, 128))
    for g in range(G):
        sub = g % 4
        for cc in range(C):
            d_ = ch[g, cc]
            for j in range(J):
                for n in range(N):
                    wu[g, j * C + cc, n * C + cc] = qp[d_, n, J - 1 - j]
                    wv[g, sub * 32 + n * C + cc, j * C + cc] = c[d_, n] * qp[d_, n, j + 1]
                for i in range(j + 1):
                    wx[g, i * C + cc, j * C + cc] = g_r[d_, j - i] + (
                        w[d_] if i == j else 0.0
                    )

    # scan multipliers: a[(sub,n,cc), t] = q_n(ch(g=sub+4t))^8
    a = np.zeros((128, 2))
    for t in range(2):
        for sub in range(4):
            for n in range(N):
                for cc in range(C):
                    a[sub * 32 + n * C + cc, t] = qp[ch[sub + 4 * t, cc], n, J]

    return (
        np.ascontiguousarray(wu.transpose(1, 0, 2).reshape(128, G * 32)).astype(np.float16),
        np.ascontiguousarray(wv.transpose(1, 0, 2).reshape(128, G * 128)).astype(np.float16),
        np.ascontiguousarray(wx.transpose(1, 0, 2).reshape(128, G * 128)).astype(np.float16),
        a.astype(np.float32),
    )


_NC_CACHE = {}


def kernel(x, delta, alpha, beta, gamma, omega):
    x = np.asarray(x, dtype=np.float32)
    delta = np.asarray(delta, dtype=np.float32)
    alpha = np.asarray(alpha, dtype=np.float32)
    beta = np.asarray(beta, dtype=np.float32)
    gamma = np.asarray(gamma, dtype=np.float32)
    omega = np.asarray(omega, dtype=np.float32)
    assert x.shape == (B, S, D)

    if "nc" not in _NC_CACHE:
        _NC_CACHE["nc"] = build_nc()
    nc = _NC_CACHE["nc"]

    xt = x.transpose(0, 2, 1)  # [B, D, S]
    in_maps = []
    for i in range(N_CORES):
        sl = slice(i * D_LOC, (i + 1) * D_LOC)
        wu, wv, wx, a = _host_params(delta, alpha, beta, gamma, omega, sl)
        # phase-major pack: [B, 128ch, S] -> [B, (j,c), g*M+m]
        xs = xt[:, sl, :].reshape(B, G, C, M, J)
        x_dev = np.ascontiguousarray(
            xs.transpose(0, 4, 2, 1, 3).reshape(B, 128, G * M)
        ).astype(np.float16)
        in_maps.append({"x": x_dev, "wu": wu, "wv": wv, "wx": wx, "a": a})

    res = run_bass_kernel_spmd(nc, in_maps, core_ids=list(range(N_CORES)))

    out = np.empty((B, S, D), dtype=np.float32)
    for i in range(N_CORES):
        sl = slice(i * D_LOC, (i + 1) * D_LOC)
        ov = res.results[i]["o"].reshape(B, J, C, G, M)
        oc = ov.transpose(0, 3, 2, 4, 1).reshape(B, D_LOC, S)  # [b, ch, t]
        out[:, :, sl] = oc.transpose(0, 2, 1).astype(np.float32)
    return out


# revision 8
# speedup vs baseline: 2.0212x; 1.1384x over previous
"""MultiHeadEMA Trainium2 Bass kernel (radix-8 blocked scan, matmul-offloaded).

Reference computation (B=4, S=8192, D=1024, N=2):
    out = silu(conv_causal(x, k) + x * omega)
    k[d, l] = sum_n c[d, n] * q[d, n]^l
    q = 1 - sigmoid(delta) * sigmoid(alpha)
    c = sigmoid(delta) * beta * gamma * sqrt(1/N)

The causal conv is a pair of first-order recurrences per channel.  On TRN2
the per-partition-scalar DVE ops (scalar_tensor_tensor / tensor_tensor_scan)
run at 1 elem/cycle with no fast modes, so the baseline that ran everything
on the Vector engine was Vector-bound at ~94% busy.  This version blocks the
recurrence by J=8 timesteps and restructures all the muladd work as
cross-partition matmuls on the otherwise-idle Tensor engine:

  - layout: partition p = (phase j in [0,8), channel c in [0,16)) per group
    of 16 channels; 8 groups cover the core's 128 channels; free dim is the
    block index m in [0, S/8).
  - u[m]   = sum_j q^{7-j} x[8m+j]          -> one matmul pass (weights wu)
  - h[m]   = q^8 h[m-1] + u[m]              -> DVE scan, 8x shorter, both
             recurrences x 4 groups stacked in one 128-partition scan
  - y[8m+j] = sum_n c_n q_n^{j+1} h_n[m-1]  -> matmul pass (weights wv)
            + sum_{i<=j} g_{j-i} x[8m+i] + w x[8m+j]   -> matmul pass (wx)
  - out = silu(y) fused on the Scalar engine, PSUM -> fp16 SBUF.

fp16 end-to-end I/O halves DMA traffic (8 MiB in + 8 MiB out per core);
weights are host-computed in fp64 and shipped as fp16 (the scan multiplier
q^8 stays fp32; the scan state is fp32 internally).  Numpy sim of this exact
quantization measures rel err 6.7e-4 vs the fp32 reference.

Sharding: D=1024 split across 8 cores (128 channels each); host packs the
phase-major fp16 layout and unpacks the result (part of shard/unshard).
"""

import math

import numpy as np

import concourse.bass as bass
import concourse.mybir as mybir
import concourse.tile as tile
from concourse import bacc
from concourse.bass_utils import run_bass_kernel_spmd

B = 4
S = 8192
D = 1024
N = 2
N_CORES = 8
D_LOC = D // N_CORES      # 128 channels per core
J = 8                     # timesteps per block (radix)
C = 16                    # channels per group
G = D_LOC // C            # 8 groups
M = S // J                # 1024 blocks per batch
SCALE = math.sqrt(1.0 / N)

F32 = mybir.dt.float32
F16 = mybir.dt.float16


def build_nc(x_bufs=4, o_bufs=3, h_bufs=2, y_bufs=2, act="Silu"):
    """Per-core Bass module (SPMD: same NEFF on all cores).

    Inputs (per core):
      x  [B, 128, G*M] f16 — phase-major shard: x[b, j*16+c, g*M+m]
                              = x_orig[b, t=8m+j, ch=16g+c]
      wu [128, G*32]   f16 — u-prep weights, lhsT per group
      wv [128, G*128]  f16 — h-combine weights, lhsT per group
      wx [128, G*128]  f16 — x-combine weights, lhsT per group
      a  [128, 2]      f32 — scan multipliers q^8 for stacked tiles A, B
    Output:
      o  [B, 128, G*M] f16 — same layout as x
    """
    nc = bacc.Bacc(
        "TRN2",
        target_bir_lowering=False,
        debug=False,
        enable_asserts=False,
        num_devices=N_CORES,
    )

    x_d = nc.dram_tensor("x", [B, 128, G * M], F16, kind="ExternalInput").ap()
    wu_d = nc.dram_tensor("wu", [128, G * 32], F16, kind="ExternalInput").ap()
    wv_d = nc.dram_tensor("wv", [128, G * 128], F16, kind="ExternalInput").ap()
    wx_d = nc.dram_tensor("wx", [128, G * 128], F16, kind="ExternalInput").ap()
    a_d = nc.dram_tensor("a", [128, 2], F32, kind="ExternalInput").ap()
    o_d = nc.dram_tensor("o", [B, 128, G * M], F16, kind="ExternalOutput").ap()

    mult = mybir.AluOpType.mult
    add = mybir.AluOpType.add
    ACT = getattr(mybir.ActivationFunctionType, act)

    with tile.TileContext(nc) as tc:
        with (
            tc.tile_pool(name="w", bufs=1) as w_pool,
            tc.tile_pool(name="x", bufs=x_bufs) as x_pool,
            tc.tile_pool(name="o", bufs=o_bufs) as o_pool,
            tc.tile_pool(name="h", bufs=h_bufs) as h_pool,
            tc.tile_pool(name="u", bufs=1, space="PSUM") as u_pool,
            tc.tile_pool(name="y", bufs=y_bufs, space="PSUM") as y_pool,
        ):
            # params ride the Scalar HWDGE ring so they overlap the first x
            # batch on the Sync ring (two independent HWDGE rings).
            wu_t = w_pool.tile([128, G * 32], F16, tag="wu")
            wv_t = w_pool.tile([128, G * 128], F16, tag="wv")
            wx_t = w_pool.tile([128, G * 128], F16, tag="wx")
            a_t = w_pool.tile([128, 2], F32, tag="a")
            nc.scalar.dma_start(out=wu_t[:], in_=wu_d[:])
            nc.scalar.dma_start(out=wv_t[:], in_=wv_d[:])
            nc.scalar.dma_start(out=wx_t[:], in_=wx_d[:])
            nc.scalar.dma_start(out=a_t[:], in_=a_d[:])

            for b in range(B):
                xb = x_pool.tile([128, G * M], F16, tag="x")
                nc.sync.dma_start(out=xb[:], in_=x_d[b])

                # u-prep: one matmul per (group, 512-col chunk) into the
                # group's 32-partition slice of the stacked PSUM tile.
                u_t = [
                    u_pool.tile([128, M], F32, tag="uA", name="uA"),
                    u_pool.tile([128, M], F32, tag="uB", name="uB"),
                ]
                for g in range(G):
                    tidx, sub = divmod(g, 4)
                    for ck in range(M // 512):
                        nc.tensor.matmul(
                            u_t[tidx][sub * 32 : sub * 32 + 32,
                                      ck * 512 : (ck + 1) * 512],
                            lhsT=wu_t[:, g * 32 : (g + 1) * 32],
                            rhs=xb[:, g * M + ck * 512 : g * M + (ck + 1) * 512],
                            start=True,
                            stop=True,
                            tile_position=(0, sub * 32),
                        )

                # stacked scans: h[m] = q^8 h[m-1] + u[m], fp32 state,
                # fp16 stored h; col 0 holds h[-1] = 0 (batches are
                # independent sequences).
                h_t = [
                    h_pool.tile([128, M + 1], F16, tag="hA", name="hA"),
                    h_pool.tile([128, M + 1], F16, tag="hB", name="hB"),
                ]
                for t in range(2):
                    nc.vector.memset(h_t[t][:, 0:1], 0.0)
                    nc.vector.tensor_tensor_scan(
                        h_t[t][:, 1 : M + 1],
                        a_t[:, t : t + 1].broadcast_to([128, M]),
                        u_t[t][:, :],
                        0.0,
                        mult,
                        add,
                    )

                # combines + fused silu; same-weight matmuls are adjacent so
                # LDWEIGHTS double-buffers cleanly, one big ACT per group.
                ob = o_pool.tile([128, G * M], F16, tag="o")
                for g in range(G):
                    hv = h_t[g // 4]
                    y = y_pool.tile([128, M], F32, tag="y")
                    for ck in range(M // 512):
                        nc.tensor.matmul(
                            y[:, ck * 512 : (ck + 1) * 512],
                            lhsT=wv_t[:, g * 128 : (g + 1) * 128],
                            rhs=hv[:, ck * 512 : (ck + 1) * 512],
                            start=True,
                            stop=False,
                        )
                    for ck in range(M // 512):
                        nc.tensor.matmul(
                            y[:, ck * 512 : (ck + 1) * 512],
                            lhsT=wx_t[:, g * 128 : (g + 1) * 128],
                            rhs=xb[:, g * M + ck * 512 : g * M + (ck + 1) * 512],
                            start=False,
                            stop=True,
                        )
                    nc.scalar.activation(
                        ob[:, g * M : (g + 1) * M],
                        y[:],
                        ACT,
                    )
                nc.scalar.dma_start(out=o_d[b], in_=ob[:])

    nc.compile()
    return nc


def _host_params(delta, alpha, beta, gamma, omega, sl):
    """Per-core weight construction (channel slice sl; fp64 math)."""
    d = delta[sl, :, 0].astype(np.float64)
    al = alpha[sl, :, 0].astype(np.float64)
    p = 1.0 / (1.0 + np.exp(-d))
    aa = 1.0 / (1.0 + np.exp(-al))
    q = 1.0 - p * aa                                     # [128, N]
    c = p * beta[sl, :, 0].astype(np.float64) * gamma[sl].astype(np.float64) * SCALE
    w = omega[sl].astype(np.float64)                     # [128]
    ch = np.arange(D_LOC).reshape(G, C)                  # ch[g, cc] = 16g+cc

    qp = q[:, :, None] ** np.arange(J + 2)[None, None, :]   # [128, N, J+2]

    # wu[g][(j,cc'), n*16+cc] = q_n(ch)^{7-j} delta_{cc,cc'}
    wu = np.zeros((G, 128, 2 * C))
    # wv[g][(sub,n,cc'), (j,cc)] = delta_{sub,g%4} c_n q_n^{j+1}
    wv = np.zeros((G, 128, 128))
    # wx[g][(i,cc'), (j,cc)] = i<j: g_{j-i};  i==j: g_0 + w
    g_r = np.einsum("dn,dnr->dr", c, qp)                 # [128, J+2]
    wx = np.zeros((G, 128, 128))
    for g in range(G):
        sub = g % 4
        for cc in range(C):
            d_ = ch[g, cc]
            for j in range(J):
                for n in range(N):
                    wu[g, j * C + cc, n * C + cc] = qp[d_, n, J - 1 - j]
                    wv[g, sub * 32 + n * C + cc, j * C + cc] = c[d_, n] * qp[d_, n, j + 1]
                for i in range(j + 1):
                    wx[g, i * C + cc, j * C + cc] = g_r[d_, j - i] + (
                        w[d_] if i == j else 0.0
                    )

    # scan multipliers: a[(sub,n,cc), t] = q_n(ch(g=sub+4t))^8
    a = np.zeros((128, 2))
    for t in range(2):
        for sub in range(4):
            for n in range(N):
                for cc in range(C):
                    a[sub * 32 + n * C + cc, t] = qp[ch[sub + 4 * t, cc], n, J]

    return (
        np.ascontiguousarray(wu.transpose(1, 0, 2).reshape(128, G * 32)).astype(np.float16),
        np.ascontiguousarray(wv.transpose(1, 0, 2).reshape(128, G * 128)).astype(np.float16),
        np.ascontiguousarray(wx.transpose(1, 0, 2).reshape(128, G * 128)).astype(np.float16),
        a.astype(np.float32),
    )


_NC_CACHE = {}


def kernel(x, delta, alpha, beta, gamma, omega):
    x = np.asarray(x, dtype=np.float32)
    delta = np.asarray(delta, dtype=np.float32)
    alpha = np.asarray(alpha, dtype=np.float32)
    beta = np.asarray(beta, dtype=np.float32)
    gamma = np.asarray(gamma, dtype=np.float32)
    omega = np.asarray(omega, dtype=np.float32)
    assert x.shape == (B, S, D)

    if "nc" not in _NC_CACHE:
        _NC_CACHE["nc"] = build_nc()
    nc = _NC_CACHE["nc"]

    xt = x.transpose(0, 2, 1)  # [B, D, S]
    in_maps = []
    for i in range(N_CORES):
        sl = slice(i * D_LOC, (i + 1) * D_LOC)
        wu, wv, wx, a = _host_params(delta, alpha, beta, gamma, omega, sl)
        # phase-major pack: [B, 128ch, S] -> [B, (j,c), g*M+m]
        xs = xt[:, sl, :].reshape(B, G, C, M, J)
        x_dev = np.ascontiguousarray(
            xs.transpose(0, 4, 2, 1, 3).reshape(B, 128, G * M)
        ).astype(np.float16)
        in_maps.append({"x": x_dev, "wu": wu, "wv": wv, "wx": wx, "a": a})

    res = run_bass_kernel_spmd(nc, in_maps, core_ids=list(range(N_CORES)))

    out = np.empty((B, S, D), dtype=np.float32)
    for i in range(N_CORES):
        sl = slice(i * D_LOC, (i + 1) * D_LOC)
        ov = res.results[i]["o"].reshape(B, J, C, G, M)
        oc = ov.transpose(0, 3, 2, 4, 1).reshape(B, D_LOC, S)  # [b, ch, t]
        out[:, :, sl] = oc.transpose(0, 2, 1).astype(np.float32)
    return out


# revision 9
# speedup vs baseline: 2.4929x; 1.2334x over previous
"""MultiHeadEMA Trainium2 Bass kernel (radix-16 blocked scan, matmul-offloaded).

Reference computation (B=4, S=8192, D=1024, N=2):
    out = silu(conv_causal(x, k) + x * omega)
    k[d, l] = sum_n c[d, n] * q[d, n]^l
    q = 1 - sigmoid(delta) * sigmoid(alpha)
    c = sigmoid(delta) * beta * gamma * sqrt(1/N)

The causal conv is a pair of first-order recurrences per channel.  On TRN2
the per-partition-scalar DVE ops (scalar_tensor_tensor / tensor_tensor_scan)
run at 1 elem/cycle with no fast modes, so a pure-Vector implementation is
Vector-bound.  This version blocks the recurrence by J=16 timesteps and
restructures all muladd work as cross-partition matmuls on the otherwise
idle Tensor engine:

  - layout: partition p = (phase j in [0,16), channel c in [0,8)) per group
    of 8 channels; 16 groups cover the core's 128 channels; free dim is the
    block index m in [0, M=S/16).
  - u[m]    = sum_j q^{15-j} x[16m+j]        -> matmul pass (weights wu)
  - h[m]    = q^16 h[m-1] + u[m]             -> DVE scan, 16x shorter, both
              recurrences x 8 groups stacked per 128-partition scan
  - y[16m+j] = sum_n c_n q_n^{j+1} h_n[m-1]  -> matmul pass (weights wv)
             + sum_{i<=j} g_{j-i} x[16m+i] + w x[16m+j]  -> matmul pass (wx)
  - out = silu(y) fused on the Scalar engine, PSUM -> fp16 SBUF.

Groups pair up (2k, 2k+1) so u-matmuls accumulate into a shared 32-partition
PSUM window (tile_position must be 32-aligned) and each ACT covers a
1024-wide y pair.  fp16 end-to-end I/O halves DMA traffic (8 MiB in + 8 MiB
out per core); weights are host-computed in fp64 and shipped as fp16; the
scan multiplier q^16 and scan state stay fp32.  Numpy sim of this exact
quantization measures rel err ~6e-4 vs the fp32 reference.

Sharding: D=1024 split across 8 cores (128 channels each); host packs the
phase-major fp16 layout and unpacks the result (part of shard/unshard).
"""

import math

import numpy as np

import concourse.bass as bass
import concourse.mybir as mybir
import concourse.tile as tile
from concourse import bacc
from concourse.bass_utils import run_bass_kernel_spmd

B = 4
S = 8192
D = 1024
N = 2
N_CORES = 8
D_LOC = D // N_CORES      # 128 channels per core
J = 16                    # timesteps per block (radix)
C = 8                     # channels per group
G = D_LOC // C            # 16 groups
M = S // J                # 512 blocks per batch
SCALE = math.sqrt(1.0 / N)

F32 = mybir.dt.float32
F16 = mybir.dt.float16


def build_nc(x_bufs=4, o_bufs=3, h_bufs=2, u_bufs=2, y_bufs=2, act="Silu"):
    """Per-core Bass module (SPMD: same NEFF on all cores).

    Inputs (per core):
      x  [B, 128, G*M] f16 — phase-major shard: x[b, j*8+c, g*M+m]
                              = x_orig[b, t=16m+j, ch=8g+c]
      wu [128, G*32]   f16 — u-prep weights, lhsT per group
      wv [128, G*128]  f16 — h-combine weights, lhsT per group
      wx [128, G*128]  f16 — x-combine weights, lhsT per group
      a  [128, 2]      f32 — scan multipliers q^16 for stacked tiles A, B
    Output:
      o  [B, 128, G*M] f16 — same layout as x
    """
    nc = bacc.Bacc(
        "TRN2",
        target_bir_lowering=False,
        debug=False,
        enable_asserts=False,
        num_devices=N_CORES,
    )

    x_d = nc.dram_tensor("x", [B, 128, G * M], F16, kind="ExternalInput").ap()
    wu_d = nc.dram_tensor("wu", [128, G * 32], F16, kind="ExternalInput").ap()
    wv_d = nc.dram_tensor("wv", [128, G * 128], F16, kind="ExternalInput").ap()
    wx_d = nc.dram_tensor("wx", [128, G * 128], F16, kind="ExternalInput").ap()
    a_d = nc.dram_tensor("a", [128, 2], F32, kind="ExternalInput").ap()
    o_d = nc.dram_tensor("o", [B, 128, G * M], F16, kind="ExternalOutput").ap()

    mult = mybir.AluOpType.mult
    add = mybir.AluOpType.add
    ACT = getattr(mybir.ActivationFunctionType, act)
    HALF = G * M // 2

    with tile.TileContext(nc) as tc:
        with (
            tc.tile_pool(name="w", bufs=1) as w_pool,
            tc.tile_pool(name="x", bufs=x_bufs) as x_pool,
            tc.tile_pool(name="o", bufs=o_bufs) as o_pool,
            tc.tile_pool(name="h", bufs=h_bufs) as h_pool,
            tc.tile_pool(name="u", bufs=u_bufs, space="PSUM") as u_pool,
            tc.tile_pool(name="y", bufs=y_bufs, space="PSUM") as y_pool,
        ):
            # params ride the Scalar HWDGE ring so they overlap the first x
            # half-batch on the Sync ring (two independent HWDGE rings).
            wu_t = w_pool.tile([128, G * 32], F16, tag="wu")
            wv_t = w_pool.tile([128, G * 128], F16, tag="wv")
            wx_t = w_pool.tile([128, G * 128], F16, tag="wx")
            a_t = w_pool.tile([128, 2], F32, tag="a")
            nc.scalar.dma_start(out=wu_t[:], in_=wu_d[:])
            nc.scalar.dma_start(out=wv_t[:], in_=wv_d[:])
            nc.scalar.dma_start(out=wx_t[:], in_=wx_d[:])
            nc.scalar.dma_start(out=a_t[:], in_=a_d[:])

            for b in range(B):
                # split the load so the first u-matmuls start after ~1 MB
                xb = x_pool.tile([128, G * M], F16, tag="x")
                nc.sync.dma_start(out=xb[:, :HALF], in_=x_d[b, :, :HALF])
                nc.sync.dma_start(out=xb[:, HALF:], in_=x_d[b, :, HALF:])

                # u-prep: group pairs (2k, 2k+1) accumulate into a shared
                # 32-partition window of the stacked PSUM tile.
                u_t = [
                    u_pool.tile([128, M], F32, tag="uA", name="uA"),
                    u_pool.tile([128, M], F32, tag="uB", name="uB"),
                ]
                for k in range(G // 2):
                    tidx, w = divmod(k, 4)
                    for half in range(2):
                        g = 2 * k + half
                        nc.tensor.matmul(
                            u_t[tidx][w * 32 : w * 32 + 32, :],
                            lhsT=wu_t[:, g * 32 : (g + 1) * 32],
                            rhs=xb[:, g * M : (g + 1) * M],
                            start=(half == 0),
                            stop=(half == 1),
                            tile_position=(0, w * 32),
                        )

                # stacked scans: h[m] = q^16 h[m-1] + u[m], fp32 state,
                # fp16 stored h; col 0 holds h[-1] = 0 (batches are
                # independent sequences).
                h_t = [
                    h_pool.tile([128, M + 1], F16, tag="hA", name="hA"),
                    h_pool.tile([128, M + 1], F16, tag="hB", name="hB"),
                ]
                for t in range(2):
                    nc.vector.memset(h_t[t][:, 0:1], 0.0)
                    nc.vector.tensor_tensor_scan(
                        h_t[t][:, 1 : M + 1],
                        a_t[:, t : t + 1].broadcast_to([128, M]),
                        u_t[t][:, :],
                        0.0,
                        mult,
                        add,
                    )

                # combines + fused silu, one 1024-wide y pair per 2 groups
                ob = o_pool.tile([128, G * M], F16, tag="o")
                for k in range(G // 2):
                    hv = h_t[k // 4]
                    y = y_pool.tile([128, 2 * M], F32, tag="y")
                    for half in range(2):
                        g = 2 * k + half
                        ys = y[:, half * M : (half + 1) * M]
                        nc.tensor.matmul(
                            ys,
                            lhsT=wv_t[:, g * 128 : (g + 1) * 128],
                            rhs=hv[:, 0:M],
                            start=True,
                            stop=False,
                        )
                        nc.tensor.matmul(
                            ys,
                            lhsT=wx_t[:, g * 128 : (g + 1) * 128],
                            rhs=xb[:, g * M : (g + 1) * M],
                            start=False,
                            stop=True,
                        )
                    nc.scalar.activation(
                        ob[:, 2 * k * M : 2 * (k + 1) * M],
                        y[:],
                        ACT,
                    )
                nc.scalar.dma_start(out=o_d[b, :, :HALF], in_=ob[:, :HALF])
                nc.scalar.dma_start(out=o_d[b, :, HALF:], in_=ob[:, HALF:])

    nc.compile()
    return nc


def _host_params(delta, alpha, beta, gamma, omega, sl):
    """Per-core weight construction (channel slice sl; fp64 math)."""
    d = delta[sl, :, 0].astype(np.float64)
    al = alpha[sl, :, 0].astype(np.float64)
    p = 1.0 / (1.0 + np.exp(-d))
    aa = 1.0 / (1.0 + np.exp(-al))
    q = 1.0 - p * aa                                     # [128, N]
    c = p * beta[sl, :, 0].astype(np.float64) * gamma[sl].astype(np.float64) * SCALE
    w = omega[sl].astype(np.float64)                     # [128]
    ch = np.arange(D_LOC).reshape(G, C)                  # ch[g, cc] = 8g+cc

    qp = q[:, :, None] ** np.arange(J + 2)[None, None, :]   # [128, N, J+2]
    g_r = np.einsum("dn,dnr->dr", c, qp)                 # [128, J+2]

    # stacked-tile partition of (g, n, cc):
    #   tile = g//8, w = (g%8)//2, half = g%2 -> p = 32w + 16*half + 8n + cc
    def stack_p(g, n, cc):
        rem = g % 8
        return 32 * (rem // 2) + 16 * (g % 2) + 8 * n + cc

    wu = np.zeros((G, 128, 32))
    wv = np.zeros((G, 128, 128))
    wx = np.zeros((G, 128, 128))
    a = np.zeros((128, 2))
    for g in range(G):
        for cc in range(C):
            d_ = ch[g, cc]
            # wu cols are window-relative: 16*half + 8n + cc
            col0 = 16 * (g % 2)
            for n in range(N):
                a[stack_p(g, n, cc), g // 8] = qp[d_, n, J]
                for j in range(J):
                    wu[g, j * C + cc, col0 + 8 * n + cc] = qp[d_, n, J - 1 - j]
                    wv[g, stack_p(g, n, cc), j * C + cc] = c[d_, n] * qp[d_, n, j + 1]
            for j in range(J):
                for i in range(j + 1):
                    wx[g, i * C + cc, j * C + cc] = g_r[d_, j - i] + (
                        w[d_] if i == j else 0.0
                    )

    return (
        np.ascontiguousarray(wu.transpose(1, 0, 2).reshape(128, G * 32)).astype(np.float16),
        np.ascontiguousarray(wv.transpose(1, 0, 2).reshape(128, G * 128)).astype(np.float16),
        np.ascontiguousarray(wx.transpose(1, 0, 2).reshape(128, G * 128)).astype(np.float16),
        a.astype(np.float32),
    )


_NC_CACHE = {}


def kernel(x, delta, alpha, beta, gamma, omega):
    x = np.asarray(x, dtype=np.float32)
    delta = np.asarray(delta, dtype=np.float32)
    alpha = np.asarray(alpha, dtype=np.float32)
    beta = np.asarray(beta, dtype=np.float32)
    gamma = np.asarray(gamma, dtype=np.float32)
    omega = np.asarray(omega, dtype=np.float32)
    assert x.shape == (B, S, D)

    if "nc" not in _NC_CACHE:
        _NC_CACHE["nc"] = build_nc()
    nc = _NC_CACHE["nc"]

    xt = x.transpose(0, 2, 1)  # [B, D, S]
    in_maps = []
    for i in range(N_CORES):
        sl = slice(i * D_LOC, (i + 1) * D_LOC)
        wu, wv, wx, a = _host_params(delta, alpha, beta, gamma, omega, sl)
        # phase-major pack: [B, 128ch, S] -> [B, (j,c), g*M+m]
        xs = xt[:, sl, :].reshape(B, G, C, M, J)
        x_dev = np.ascontiguousarray(
            xs.transpose(0, 4, 2, 1, 3).reshape(B, 128, G * M)
        ).astype(np.float16)
        in_maps.append({"x": x_dev, "wu": wu, "wv": wv, "wx": wx, "a": a})

    res = run_bass_kernel_spmd(nc, in_maps, core_ids=list(range(N_CORES)))

    out = np.empty((B, S, D), dtype=np.float32)
    for i in range(N_CORES):
        sl = slice(i * D_LOC, (i + 1) * D_LOC)
        ov = res.results[i]["o"].reshape(B, J, C, G, M)
        oc = ov.transpose(0, 3, 2, 4, 1).reshape(B, D_LOC, S)  # [b, ch, t]
        out[:, :, sl] = oc.transpose(0, 2, 1).astype(np.float32)
    return out
